# revision 25
# baseline (speedup 1.0000x reference)
"""Causal self-attention (B=4, T=2048, C=1024, 16 heads) on 8 trn2 NeuronCores.

Sharding: core (b, hg) handles batch b (4) x head-group hg (2 groups of 8 heads).
Each core computes QKV projection for its batch restricted to its 8 heads,
flash-style causal attention, and the output projection restricted to its
heads' rows of w_proj -> a partial [T, C] output. Host sums the two partials
per batch (tensor-parallel unshard) and concatenates batches.

Key layout choices (all bf16 matmul inputs, fp32 PSUM accumulation):
  - x is fed pre-transposed per batch: xT [C, T], so Q^T/K^T [d, t] come
    straight out of the QKV matmuls (lhsT = w slice, rhs = xT).
  - Scores are computed TRANSPOSED: S^T[tk, q] = matmul with lhsT = K^T chunk,
    rhs = Q^T chunk; the pair's two heads sit on partition halves 0:64/64:128
    so their score matmuls row-tile onto disjoint PE row groups.
  - V carries an appended ones-column, so the A@V matmul (lhsT=[V|1], rhs=P^T)
    yields y~^T = [64 weighted-V rows | l row] x q in one [65, 512] pass.
    Since d is already on partitions, y~^T IS y^T unnormalized: the softmax
    division is done per q-COLUMN by broadcasting 1/l (DVE reciprocal of the
    l row -> GpSimd partition_broadcast -> DVE multiply) straight into
    y2t [128 ch, T] pair-chunks. No PE transposes anywhere.
  - Causal masking: block-skipping, matmul column ranges narrowed to the valid
    q-range on diagonal slots, exp written only to the valid range of a
    dedicated diag buffer (whose sub-diagonal zeros are written once), and one
    [128,128] staircase mask multiplied into the true diagonal blocks.
  - Next pair's QKV projection matmuls are interleaved into the attention
    c-loop as PE filler while ScalarE works through the exps; pair 3
    interleaves the output-projection tail the same way. QK projection runs
    t4-pair-inner (same stationary weights for 2 consecutive matmuls) and the
    output projection co-inner (same y^T chunk for 2 matmuls) to cut
    LDWEIGHTS pressure.
"""

import numpy as np
import ml_dtypes

B, T, C, H, D = 4, 2048, 1024, 16, 64
P = 128
TC = T // P          # 16 t-chunks of 128
KC = C // P          # 8 contraction chunks of 128
NPAIR = 4            # head pairs per core (8 local heads)
SCALE = 0.125        # 1/sqrt(64)

_CACHE = {}
LAST_RESULT = None   # BassKernelResults of the most recent run (for test.py)

BF16 = ml_dtypes.bfloat16


def _build_program():
    import concourse.tile as tile
    import concourse.mybir as mybir
    from concourse import bacc

    dt = mybir.dt
    AF = mybir.ActivationFunctionType
    ALU = mybir.AluOpType

    # Bias the ACT-table placement pass so Exp and Ln both resolve to the
    # combined natural_log_exp_and_others set (index preserved): the pass
    # maps each function to the FIRST listed set containing it, and mixing
    # Exp/Ln would otherwise reload tables (~2.7us) on every transition.
    # Only the pass's view of set membership is masked; runtime table
    # contents and set ids are untouched.
    if not getattr(bacc, "_act_tables_patched", False):
        _orig_tables = bacc.get_activation_tables

        def _patched_tables(module_arch):
            out = {}
            for name, funcs in _orig_tables(module_arch).items():
                funcs = set(funcs)
                if name in ("exp_and_others", "natural_log"):
                    funcs.discard(AF.Exp)
                    funcs.discard(AF.Ln)
                out[name] = funcs
            return out

        bacc.get_activation_tables = _patched_tables
        bacc._act_tables_patched = True

    nc = bacc.Bacc("TRN2", target_bir_lowering=False, debug=False, num_devices=8)

    # ---- DRAM I/O ----
    xT_d = nc.dram_tensor("xT", [C, T], dt.bfloat16, kind="ExternalInput").ap()
    wqk_d = nc.dram_tensor("wqk", [C, 1024], dt.bfloat16, kind="ExternalInput").ap()
    wv_d = nc.dram_tensor("wv", [C, 512], dt.bfloat16, kind="ExternalInput").ap()
    wproj_d = nc.dram_tensor("wproj", [512, C], dt.bfloat16, kind="ExternalInput").ap()
    bqk_d = nc.dram_tensor("bqk", [P, 8], dt.float32, kind="ExternalInput").ap()
    bv_d = nc.dram_tensor("bv", [P, 512], dt.float32, kind="ExternalInput").ap()
    bproj_d = nc.dram_tensor("bproj", [P, C], dt.float32, kind="ExternalInput").ap()
    dmask_d = nc.dram_tensor("dmask", [P, P], dt.bfloat16, kind="ExternalInput").ap()
    out_d = nc.dram_tensor("out", [T, C], dt.bfloat16, kind="ExternalOutput").ap()

    with tile.TileContext(nc) as tc:
        with (
            tc.tile_pool(name="const", bufs=1) as cp,
            tc.tile_pool(name="outp", bufs=4) as op_pool,
            tc.tile_pool(name="small", bufs=3) as sp,
            tc.tile_pool(name="lnp", bufs=3) as ln_pool,
            tc.tile_pool(name="lbp", bufs=2) as lb_pool,
            tc.tile_pool(name="ytp", bufs=3) as yt_pool,
            tc.tile_pool(name="psqk", bufs=2, space="PSUM") as psqk_pool,
            tc.tile_pool(name="psmm", bufs=2, space="PSUM") as psmm_pool,
            tc.tile_pool(name="psqp", bufs=2, space="PSUM") as psqp_pool,
        ):
            # ---- static SBUF tensors ----
            xT_s = cp.tile([P, KC, T], dt.bfloat16, name="xT_s")
            wqk_s = cp.tile([P, KC, 1024], dt.bfloat16, name="wqk_s")
            wv_s = cp.tile([P, KC, 512], dt.bfloat16, name="wv_s")
            wproj_s = cp.tile([P, 4, C], dt.bfloat16, name="wproj_s")
            bqk_s = cp.tile([P, 8], dt.float32, name="bqk_s")
            bv_s = cp.tile([P, 512], dt.float32, name="bv_s")
            bproj_s = cp.tile([P, C], dt.float32, name="bproj_s")
            dmask_s = cp.tile([P, P], dt.bfloat16, name="dmask_s")
            qt_s = cp.tile([P, NPAIR, T], dt.bfloat16, name="qt_s")
            kt_s = cp.tile([P, NPAIR, T], dt.bfloat16, name="kt_s")
            # head stride 72 (144B) keeps each [V|1] slice 16B-aligned for LDW
            v_s = cp.tile([P, TC, 8, 72], dt.bfloat16, name="v_s")   # [t, tc, head, V|1|pad]
            y2t_s = cp.tile([P, NPAIR, T], dt.bfloat16, name="y2t_s")  # y^T [ch, t]
            pt_s = cp.tile([P, 12, 2, 512], dt.bfloat16, name="pt_s")   # exp(S^T) off-diag [slot,hh,q]
            # diag slots, flat [hh0 q | hh1 q-shifted]: hh1's valid region is
            # written at 512..1024-q0 so one contiguous exp covers both heads
            ptd_s = cp.tile([P, 4, 1024], dt.bfloat16, name="ptd_s")

            # ---- input DMAs (weights for the first compute first) ----
            xT_src = xT_d.rearrange("(o p) t -> p o t", p=P)
            wv_src = wv_d.rearrange("(o p) m -> p o m", p=P)
            wqk_src = wqk_d.rearrange("(o p) m -> p o m", p=P)
            nc.sync.dma_start(wv_s[:, 0:2, :], wv_src[:, 0:2, :])
            nc.sync.dma_start(wv_s[:, 2:4, :], wv_src[:, 2:4, :])
            nc.sync.dma_start(xT_s[:, :, 0:128], xT_src[:, :, 0:128])
            nc.sync.dma_start(wv_s[:, 4:6, :], wv_src[:, 4:6, :])
            nc.sync.dma_start(wv_s[:, 6:8, :], wv_src[:, 6:8, :])
            nc.sync.dma_start(xT_s[:, :, 128:256], xT_src[:, :, 128:256])
            nc.sync.dma_start(bv_s[:], bv_d)
            nc.sync.dma_start(xT_s[:, :, 256:512], xT_src[:, :, 256:512])
            nc.sync.dma_start(wqk_s[:, 0:4, :], wqk_src[:, 0:4, :])
            nc.sync.dma_start(xT_s[:, :, 512:768], xT_src[:, :, 512:768])
            nc.sync.dma_start(wqk_s[:, 4:8, :], wqk_src[:, 4:8, :])
            nc.sync.dma_start(bqk_s[:], bqk_d)
            for q8 in range(3, 8):
                nc.sync.dma_start(xT_s[:, :, 256 * q8:256 * (q8 + 1)],
                                  xT_src[:, :, 256 * q8:256 * (q8 + 1)])
            nc.sync.dma_start(dmask_s[:], dmask_d)
            nc.sync.dma_start(wproj_s[:], wproj_d.rearrange("(o p) m -> p o m", p=P))
            nc.sync.dma_start(bproj_s[:], bproj_d)

            # ones column of V~; zero the diag P^T buffer once (sub-diagonal
            # regions are never written by the partial exps, so zeros persist)
            nc.vector.memset(v_s[:, :, :, 64:65], 1.0)
            nc.vector.memset(ptd_s[:], 0.0)

            # ---- V projection: v[t, ch] for all 8 heads (512 cols) ----
            for tcx in range(TC):
                psv = psmm_pool.tile([P, 512], dt.float32, name="psv", tag="mm")
                for k in range(KC):
                    nc.tensor.matmul(psv[:, :],
                                     xT_s[:, k, P * tcx:P * (tcx + 1)],
                                     wv_s[:, k, :],
                                     start=(k == 0), stop=(k == KC - 1))
                nc.vector.tensor_add(
                    out=v_s[:, tcx, :, 0:64],
                    in0=psv[:, :].rearrange("a (h d) -> a h d", h=8),
                    in1=bv_s[:, :].rearrange("a (h d) -> a h d", h=8),
                )

            # ---- helper emitters ----
            def pool_tag(pool):
                return "mm" if pool is psmm_pool else "qp"

            def qkproj_half(m, th, pool):
                """Two [128 out-ch, 512 t] tiles of the Q^T/K^T projection,
                k-inner with both t4 tiles per k so each stationary weight
                chunk serves 2 consecutive matmuls."""
                dst = qt_s if m < 4 else kt_s
                t4s = (2 * th, 2 * th + 1)
                pss = [pool.tile([P, 512], dt.float32, name=f"psq{i}",
                                 tag=pool_tag(pool)) for i in range(2)]
                for k in range(KC):
                    for i, t4 in enumerate(t4s):
                        nc.tensor.matmul(pss[i][:, :],
                                         wqk_s[:, k, P * m:P * (m + 1)],
                                         xT_s[:, k, 512 * t4:512 * (t4 + 1)],
                                         start=(k == 0), stop=(k == KC - 1))
                for i, t4 in enumerate(t4s):
                    nc.vector.tensor_scalar(
                        out=dst[:, m % 4, 512 * t4:512 * (t4 + 1)],
                        in0=pss[i][:, :], scalar1=bqk_s[:, m:m + 1], scalar2=None,
                        op0=ALU.add)

            def proj_chunk(tcx, pool):
                """Output projection for one 128-row t-chunk, co-inner so each
                y^T chunk serves 2 consecutive matmuls."""
                pss = [pool.tile([P, 512], dt.float32, name=f"psp{co}",
                                 tag=pool_tag(pool)) for co in range(2)]
                for cc in range(4):
                    for co in range(2):
                        nc.tensor.matmul(pss[co][:, :],
                                         y2t_s[:, cc, P * tcx:P * (tcx + 1)],
                                         wproj_s[:, cc, 512 * co:512 * (co + 1)],
                                         start=(cc == 0), stop=(cc == 3))
                for co in range(2):
                    ot = op_pool.tile([P, 512], dt.bfloat16, name="ot", tag="ot")
                    nc.vector.tensor_add(out=ot[:, :], in0=pss[co][:, :],
                                         in1=bproj_s[:, 512 * co:512 * (co + 1)])
                    nc.sync.dma_start(
                        out_d[P * tcx:P * (tcx + 1), 512 * co:512 * (co + 1)], ot[:, :])

            # ---- pair 0 projection upfront; later pairs interleave ----
            for m in (0, 4):
                for th in range(2):
                    qkproj_half(m, th, psqp_pool)

            for pair in range(NPAIR):
                nxt = ([(m, th) for m in (pair + 1, 5 + pair) for th in range(2)]
                       if pair < NPAIR - 1 else [])
                for ci, c in enumerate(range(4)):   # q chunk of 512
                    # A@V psum pool alternates by block parity so block c's
                    # normalization chain has a whole block of slack before
                    # its banks are reused; filler/proj use the other pool
                    av_pool = psmm_pool if c % 2 == 0 else psqp_pool
                    other_pool = psqp_pool if c % 2 == 0 else psmm_pool
                    # pair 3 has no QK filler; spread the previous block's
                    # output projection across its scores phase instead
                    if pair == NPAIR - 1 and c > 0:
                        nslots = 4 * c + 4
                        pj = {(i + 1) * nslots // 5: 4 * (c - 1) + i
                              for i in range(3)}
                        late_proj = 4 * (c - 1) + 3   # odd tcx -> psqp pool
                    else:
                        pj = {}
                        late_proj = None
                    fill_units = list(nxt[ci:ci + 1])
                    for j in range(4 * c + 4):          # tk chunk (slot)
                        r = j - 4 * c                   # >= 0 on diagonal slots
                        q0 = P * r if r >= 0 else 0     # skip masked cols
                        psS = psqk_pool.tile([P, 1024], dt.float32, name="psS",
                                             tag="psqk")
                        for hh in (0, 1):
                            base = 64 * hh
                            # hh1's diag output is left-shifted by q0 so the
                            # two heads' valid score regions abut at 512
                            nc.tensor.matmul(
                                psS[:, q0:512] if hh == 0
                                else psS[:, 512:1024 - q0],
                                kt_s[base:base + 64, pair, P * j:P * (j + 1)],
                                qt_s[base:base + 64, pair,
                                     512 * c + q0:512 * (c + 1)],
                                start=True, stop=True)
                        # exp( S^T * scale ), fp32 psum -> bf16 sbuf
                        if r < 0:
                            nc.scalar.activation(pt_s[:, j, :, :], psS[:, :],
                                                 AF.Exp, scale=SCALE)
                        else:
                            nc.scalar.activation(
                                ptd_s[:, r, q0:1024 - q0],
                                psS[:, q0:1024 - q0],
                                AF.Exp, scale=SCALE)
                            # staircase mask on each head's true diag block
                            for hh in (0, 1):
                                d0 = q0 if hh == 0 else 512
                                nc.vector.tensor_tensor(
                                    out=ptd_s[:, r, d0:d0 + P],
                                    in0=ptd_s[:, r, d0:d0 + P],
                                    in1=dmask_s[:, :], op=ALU.mult)
                        if j in pj:
                            proj_chunk(pj[j], other_pool)

                    # [V | 1]^T @ P^T per head: y~^T [65, 512] = unnormalized
                    # y^T rows + the softmax-denominator l row. Normalize by
                    # broadcasting 1/l across partitions (GpSimd) and multiply
                    # straight into the y^T pair-chunk buffer -- no transposes.
                    # The QK filler (or pair 3's 4th proj chunk) is emitted
                    # inside hh0's accumulation a few slots before the end,
                    # where the PE would otherwise stall on the last diag exps.
                    for hh in (0, 1):
                        nj = 4 * c + 4
                        if hh == 0 and nj <= 4 and fill_units:
                            for (m, th) in fill_units:
                                qkproj_half(m, th, other_pool)
                            fill_units = []
                        psyt = av_pool.tile([P, 512], dt.float32,
                                            name="psyt",
                                            tag=pool_tag(av_pool))
                        for j in range(nj):
                            r = j - 4 * c
                            if r < 0:
                                rhs = pt_s[:, j, hh, :]
                                out = psyt[0:65, :]
                            else:
                                # diagonal slot: only columns q >= 128r live
                                # (hh1's region sits left-shifted at 512..)
                                rhs = (ptd_s[:, r, P * r:512] if hh == 0
                                       else ptd_s[:, r, 512:1024 - P * r])
                                out = psyt[0:65, P * r:]
                            nc.tensor.matmul(
                                out, v_s[:, j, 2 * pair + hh, 0:65], rhs,
                                start=(j == 0), stop=(j == nj - 1))
                            if hh == 0 and j == nj - 5:
                                if fill_units:
                                    for (m, th) in fill_units:
                                        qkproj_half(m, th, other_pool)
                                    fill_units = []
                                elif late_proj is not None:
                                    proj_chunk(late_proj, other_pool)
                                    late_proj = None
                        # 1/l on ScalarE as exp(-ln(l)) (DVE RECIPROCAL is
                        # ~6.5 cyc/elem serial in the free dim), GpSimd
                        # partition-broadcast, DVE multiply into the y^T chunk.
                        lnl = ln_pool.tile([1, 512], dt.float32, name="lnl",
                                           tag="lnl")
                        nc.scalar.activation(lnl[:, :], psyt[64:65, :], AF.Ln)
                        linv = sp.tile([1, 512], dt.bfloat16, name="linv",
                                       tag="linv")
                        nc.scalar.activation(linv[:, :], lnl[:, :], AF.Exp,
                                             scale=-1.0)
                        lb = lb_pool.tile([64, 512], dt.bfloat16, name="lb",
                                          tag="lb")
                        nc.gpsimd.partition_broadcast(lb[:, :], linv[:, :],
                                                      channels=64)
                        nc.vector.tensor_tensor(
                            out=y2t_s[64 * hh:64 * (hh + 1), pair,
                                      512 * c:512 * (c + 1)],
                            in0=psyt[0:64, :], in1=lb[:, :], op=ALU.mult)

                # last block's output projection (nothing left to hide
                # behind); block 3's A@V used psqp, so run these on psmm
                # to avoid waiting on its normalization chain
                if pair == NPAIR - 1:
                    for qi_loc in range(4):
                        proj_chunk(12 + qi_loc, psmm_pool)

    nc.compile()
    return nc


def _prep_inputs(x, w_attn, b_attn, w_proj, b_proj):
    """Host-side shard prep: per-core input dicts (core ci = b*2 + hg)."""
    x = np.asarray(x, dtype=np.float32)
    w_attn = np.asarray(w_attn, dtype=np.float32)
    b_attn = np.asarray(b_attn, dtype=np.float32)
    w_proj = np.asarray(w_proj, dtype=np.float32)
    b_proj = np.asarray(b_proj, dtype=np.float32)

    # diagonal staircase mask [tk, q]: valid iff q >= tk
    dmask = (np.arange(P)[None, :] >= np.arange(P)[:, None]).astype(BF16)

    in_maps = []
    for b in range(B):
        xT = np.ascontiguousarray(x[b].T).astype(BF16)       # [C, T]
        for hg in range(2):
            lo = hg * 512
            wqk = np.concatenate(
                [w_attn[:, lo:lo + 512], w_attn[:, 1024 + lo:1024 + lo + 512]],
                axis=1).astype(BF16)                          # [C, 1024]
            wv = w_attn[:, 2048 + lo:2048 + lo + 512].astype(BF16)
            wproj = w_proj[lo:lo + 512, :].astype(BF16)       # [512, C]
            bqk = np.stack(
                [b_attn[lo + P * m:lo + P * (m + 1)] for m in range(4)] +
                [b_attn[1024 + lo + P * m:1024 + lo + P * (m + 1)] for m in range(4)],
                axis=1).astype(np.float32)                    # [128, 8]
            bv = np.broadcast_to(b_attn[2048 + lo:2048 + lo + 512],
                                 (P, 512)).astype(np.float32)
            bp = b_proj if hg == 0 else np.zeros_like(b_proj)
            bproj = np.broadcast_to(bp, (P, C)).astype(np.float32)
            in_maps.append({
                "xT": xT, "wqk": wqk, "wv": wv, "wproj": wproj,
                "bqk": np.ascontiguousarray(bqk), "bv": np.ascontiguousarray(bv),
                "bproj": np.ascontiguousarray(bproj),
                "dmask": np.ascontiguousarray(dmask),
            })
    return in_maps


def kernel(x, w_attn, b_attn, w_proj, b_proj):
    global LAST_RESULT
    from concourse.bass_utils import run_bass_kernel_spmd

    if "nc" not in _CACHE:
        _CACHE["nc"] = _build_program()
    nc = _CACHE["nc"]

    in_maps = _prep_inputs(x, w_attn, b_attn, w_proj, b_proj)
    res = run_bass_kernel_spmd(nc, in_maps, core_ids=list(range(8)))
    LAST_RESULT = res

    out = np.zeros((B, T, C), dtype=np.float32)
    for b in range(B):
        out[b] = (res.results[2 * b]["out"].astype(np.float32) +
                  res.results[2 * b + 1]["out"].astype(np.float32))
    return out


# revision 26
# speedup vs baseline: 1.0768x; 1.0768x over previous
"""Causal self-attention (B=4, T=2048, C=1024, 16 heads) on 8 trn2 NeuronCores.

Sharding: core (b, hg) handles batch b (4) x head-group hg (2 groups of 8 heads).
Each core computes QKV projection for its batch restricted to its 8 heads,
flash-style causal attention, and the output projection restricted to its
heads' rows of w_proj -> a partial [T, C] output. Host sums the two partials
per batch (tensor-parallel unshard) and concatenates batches.

Key layout choices (all bf16 matmul inputs, fp32 PSUM accumulation):
  - x is fed pre-transposed per batch: xT [C, T], so Q^T/K^T [d, t] come
    straight out of the QKV matmuls (lhsT = w slice, rhs = xT).
  - Scores are computed TRANSPOSED: S^T[tk, q] = matmul with lhsT = K^T chunk,
    rhs = Q^T chunk; the pair's two heads sit on partition halves 0:64/64:128
    so their score matmuls row-tile onto disjoint PE row groups.
  - V carries an appended ones-column, so the A@V matmul (lhsT=[V|1], rhs=P^T)
    yields y~^T = [64 weighted-V rows | l row] x q in one [65, 512] pass.
    Since d is already on partitions, y~^T IS y^T unnormalized: the softmax
    division is done per q-COLUMN by broadcasting 1/l (DVE reciprocal of the
    l row -> GpSimd partition_broadcast -> DVE multiply) straight into
    y2t [128 ch, T] pair-chunks. No PE transposes anywhere.
  - Causal masking: block-skipping, matmul column ranges narrowed to the valid
    q-range on diagonal slots, exp written only to the valid range of a
    dedicated diag buffer (whose sub-diagonal zeros are written once), and one
    [128,128] staircase mask multiplied into the true diagonal blocks.
  - Next pair's QKV projection matmuls are interleaved into the attention
    c-loop as PE filler while ScalarE works through the exps; pair 3
    interleaves the output-projection tail the same way. QK projection runs
    t4-pair-inner (same stationary weights for 2 consecutive matmuls) and the
    output projection co-inner (same y^T chunk for 2 matmuls) to cut
    LDWEIGHTS pressure.
"""

import numpy as np
import ml_dtypes

B, T, C, H, D = 4, 2048, 1024, 16, 64
P = 128
TC = T // P          # 16 t-chunks of 128
KC = C // P          # 8 contraction chunks of 128
NPAIR = 4            # head pairs per core (8 local heads)
SCALE = 0.125        # 1/sqrt(64)

_CACHE = {}
LAST_RESULT = None   # BassKernelResults of the most recent run (for test.py)

BF16 = ml_dtypes.bfloat16


def _build_program():
    import concourse.tile as tile
    import concourse.mybir as mybir
    from concourse import bacc

    dt = mybir.dt
    AF = mybir.ActivationFunctionType
    ALU = mybir.AluOpType

    # Bias the ACT-table placement pass so Exp and Ln both resolve to the
    # combined natural_log_exp_and_others set (index preserved): the pass
    # maps each function to the FIRST listed set containing it, and mixing
    # Exp/Ln would otherwise reload tables (~2.7us) on every transition.
    # Only the pass's view of set membership is masked; runtime table
    # contents and set ids are untouched.
    if not getattr(bacc, "_act_tables_patched", False):
        _orig_tables = bacc.get_activation_tables

        def _patched_tables(module_arch):
            out = {}
            for name, funcs in _orig_tables(module_arch).items():
                funcs = set(funcs)
                if name in ("exp_and_others", "natural_log"):
                    funcs.discard(AF.Exp)
                    funcs.discard(AF.Ln)
                out[name] = funcs
            return out

        bacc.get_activation_tables = _patched_tables
        bacc._act_tables_patched = True

    nc = bacc.Bacc("TRN2", target_bir_lowering=False, debug=False, num_devices=8)

    # ---- DRAM I/O ----
    xT_d = nc.dram_tensor("xT", [C, T], dt.bfloat16, kind="ExternalInput").ap()
    wqk_d = nc.dram_tensor("wqk", [C, 1024], dt.bfloat16, kind="ExternalInput").ap()
    wv_d = nc.dram_tensor("wv", [C, 512], dt.bfloat16, kind="ExternalInput").ap()
    wproj_d = nc.dram_tensor("wproj", [512, C], dt.bfloat16, kind="ExternalInput").ap()
    bqk_d = nc.dram_tensor("bqk", [P, 8], dt.float32, kind="ExternalInput").ap()
    bv_d = nc.dram_tensor("bv", [P, 512], dt.float32, kind="ExternalInput").ap()
    bproj_d = nc.dram_tensor("bproj", [P, C], dt.float32, kind="ExternalInput").ap()
    dmask_d = nc.dram_tensor("dmask", [P, P], dt.bfloat16, kind="ExternalInput").ap()
    out_d = nc.dram_tensor("out", [T, C], dt.bfloat16, kind="ExternalOutput").ap()

    with tile.TileContext(nc) as tc:
        with (
            tc.tile_pool(name="const", bufs=1) as cp,
            tc.tile_pool(name="outp", bufs=4) as op_pool,
            tc.tile_pool(name="small", bufs=3) as sp,
            tc.tile_pool(name="lnp", bufs=3) as ln_pool,
            tc.tile_pool(name="lbp", bufs=2) as lb_pool,
            tc.tile_pool(name="ytp", bufs=3) as yt_pool,
            tc.tile_pool(name="psqk", bufs=2, space="PSUM") as psqk_pool,
            tc.tile_pool(name="psmm", bufs=2, space="PSUM") as psmm_pool,
            tc.tile_pool(name="psqp", bufs=2, space="PSUM") as psqp_pool,
        ):
            # ---- static SBUF tensors ----
            xT_s = cp.tile([P, KC, T], dt.bfloat16, name="xT_s")
            wqk_s = cp.tile([P, KC, 1024], dt.bfloat16, name="wqk_s")
            wv_s = cp.tile([P, KC, 512], dt.bfloat16, name="wv_s")
            wproj_s = cp.tile([P, 4, C], dt.bfloat16, name="wproj_s")
            bqk_s = cp.tile([P, 8], dt.float32, name="bqk_s")
            bv_s = cp.tile([P, 512], dt.float32, name="bv_s")
            bproj_s = cp.tile([P, C], dt.float32, name="bproj_s")
            dmask_s = cp.tile([P, P], dt.bfloat16, name="dmask_s")
            qt_s = cp.tile([P, NPAIR, T], dt.bfloat16, name="qt_s")
            kt_s = cp.tile([P, NPAIR, T], dt.bfloat16, name="kt_s")
            # head stride 72 (144B) keeps each [V|1] slice 16B-aligned for LDW
            v_s = cp.tile([P, TC, 8, 72], dt.bfloat16, name="v_s")   # [t, tc, head, V|1|pad]
            y2t_s = cp.tile([P, NPAIR, T], dt.bfloat16, name="y2t_s")  # y^T [ch, t]
            pt_s = cp.tile([P, 12, 2, 512], dt.bfloat16, name="pt_s")   # exp(S^T) off-diag [slot,hh,q]
            # diag slots, flat [hh0 q | hh1 q-shifted]: hh1's valid region is
            # written at 512..1024-q0 so one contiguous exp covers both heads
            ptd_s = cp.tile([P, 4, 1024], dt.bfloat16, name="ptd_s")

            # ---- input DMAs (weights for the first compute first) ----
            xT_src = xT_d.rearrange("(o p) t -> p o t", p=P)
            wv_src = wv_d.rearrange("(o p) m -> p o m", p=P)
            wqk_src = wqk_d.rearrange("(o p) m -> p o m", p=P)
            nc.sync.dma_start(wv_s[:, 0:2, :], wv_src[:, 0:2, :])
            nc.sync.dma_start(wv_s[:, 2:4, :], wv_src[:, 2:4, :])
            nc.sync.dma_start(xT_s[:, :, 0:128], xT_src[:, :, 0:128])
            nc.sync.dma_start(wv_s[:, 4:6, :], wv_src[:, 4:6, :])
            nc.sync.dma_start(wv_s[:, 6:8, :], wv_src[:, 6:8, :])
            nc.sync.dma_start(xT_s[:, :, 128:256], xT_src[:, :, 128:256])
            nc.sync.dma_start(bv_s[:], bv_d)
            nc.sync.dma_start(xT_s[:, :, 256:512], xT_src[:, :, 256:512])
            nc.sync.dma_start(wqk_s[:, 0:4, :], wqk_src[:, 0:4, :])
            nc.sync.dma_start(xT_s[:, :, 512:768], xT_src[:, :, 512:768])
            nc.sync.dma_start(wqk_s[:, 4:8, :], wqk_src[:, 4:8, :])
            nc.sync.dma_start(bqk_s[:], bqk_d)
            for q8 in range(3, 8):
                nc.sync.dma_start(xT_s[:, :, 256 * q8:256 * (q8 + 1)],
                                  xT_src[:, :, 256 * q8:256 * (q8 + 1)])
            nc.sync.dma_start(dmask_s[:], dmask_d)
            nc.sync.dma_start(wproj_s[:], wproj_d.rearrange("(o p) m -> p o m", p=P))
            nc.sync.dma_start(bproj_s[:], bproj_d)

            # ones column of V~; zero the diag P^T buffer once (sub-diagonal
            # regions are never written by the partial exps, so zeros persist)
            nc.vector.memset(v_s[:, :, :, 64:65], 1.0)
            nc.vector.memset(ptd_s[:], 0.0)

            # ---- V projection: v[t, ch] for all 8 heads (512 cols) ----
            for tcx in range(TC):
                psv = psmm_pool.tile([P, 512], dt.float32, name="psv", tag="mm")
                for k in range(KC):
                    nc.tensor.matmul(psv[:, :],
                                     xT_s[:, k, P * tcx:P * (tcx + 1)],
                                     wv_s[:, k, :],
                                     start=(k == 0), stop=(k == KC - 1))
                nc.vector.tensor_add(
                    out=v_s[:, tcx, :, 0:64],
                    in0=psv[:, :].rearrange("a (h d) -> a h d", h=8),
                    in1=bv_s[:, :].rearrange("a (h d) -> a h d", h=8),
                )

            # ---- helper emitters ----
            def pool_tag(pool):
                return "mm" if pool is psmm_pool else "qp"

            def qkproj_half(m, th, pool):
                """Two [128 out-ch, 512 t] tiles of the Q^T/K^T projection,
                k-inner with both t4 tiles per k so each stationary weight
                chunk serves 2 consecutive matmuls."""
                dst = qt_s if m < 4 else kt_s
                t4s = (2 * th, 2 * th + 1)
                pss = [pool.tile([P, 512], dt.float32, name=f"psq{i}",
                                 tag=pool_tag(pool)) for i in range(2)]
                for k in range(KC):
                    for i, t4 in enumerate(t4s):
                        nc.tensor.matmul(pss[i][:, :],
                                         wqk_s[:, k, P * m:P * (m + 1)],
                                         xT_s[:, k, 512 * t4:512 * (t4 + 1)],
                                         start=(k == 0), stop=(k == KC - 1))
                for i, t4 in enumerate(t4s):
                    nc.vector.tensor_scalar(
                        out=dst[:, m % 4, 512 * t4:512 * (t4 + 1)],
                        in0=pss[i][:, :], scalar1=bqk_s[:, m:m + 1], scalar2=None,
                        op0=ALU.add)

            def proj_chunk(tcx, pool):
                """Output projection for one 128-row t-chunk, co-inner so each
                y^T chunk serves 2 consecutive matmuls."""
                pss = [pool.tile([P, 512], dt.float32, name=f"psp{co}",
                                 tag=pool_tag(pool)) for co in range(2)]
                for cc in range(4):
                    for co in range(2):
                        nc.tensor.matmul(pss[co][:, :],
                                         y2t_s[:, cc, P * tcx:P * (tcx + 1)],
                                         wproj_s[:, cc, 512 * co:512 * (co + 1)],
                                         start=(cc == 0), stop=(cc == 3))
                for co in range(2):
                    ot = op_pool.tile([P, 512], dt.bfloat16, name="ot", tag="ot")
                    nc.vector.tensor_add(out=ot[:, :], in0=pss[co][:, :],
                                         in1=bproj_s[:, 512 * co:512 * (co + 1)])
                    nc.sync.dma_start(
                        out_d[P * tcx:P * (tcx + 1), 512 * co:512 * (co + 1)], ot[:, :])

            # ---- pair 0 projection upfront; later pairs interleave ----
            for m in (0, 4):
                for th in range(2):
                    qkproj_half(m, th, psqp_pool)

            for pair in range(NPAIR):
                nxt = ([(m, th) for m in (pair + 1, 5 + pair) for th in range(2)]
                       if pair < NPAIR - 1 else [])
                for ci, c in enumerate(range(4)):   # q chunk of 512
                    # pair 3 has no QK filler; spread the previous block's
                    # output projection across its scores phase instead
                    if pair == NPAIR - 1 and c > 0:
                        nslots = 4 * c + 4
                        pj = {(i + 1) * nslots // 5: 4 * (c - 1) + i
                              for i in range(3)}
                        late_proj = 4 * (c - 1) + 3   # odd tcx -> psqp pool
                    else:
                        pj = {}
                        late_proj = None
                    fill_units = list(nxt[ci:ci + 1])
                    for j in range(4 * c + 4):          # tk chunk (slot)
                        r = j - 4 * c                   # >= 0 on diagonal slots
                        q0 = P * r if r >= 0 else 0     # skip masked cols
                        psS = psqk_pool.tile([P, 1024], dt.float32, name="psS",
                                             tag="psqk")
                        for hh in (0, 1):
                            base = 64 * hh
                            # hh1's diag output is left-shifted by q0 so the
                            # two heads' valid score regions abut at 512
                            nc.tensor.matmul(
                                psS[:, q0:512] if hh == 0
                                else psS[:, 512:1024 - q0],
                                kt_s[base:base + 64, pair, P * j:P * (j + 1)],
                                qt_s[base:base + 64, pair,
                                     512 * c + q0:512 * (c + 1)],
                                start=True, stop=True)
                        # exp( S^T * scale ), fp32 psum -> bf16 sbuf
                        if r < 0:
                            nc.scalar.activation(pt_s[:, j, :, :], psS[:, :],
                                                 AF.Exp, scale=SCALE)
                        else:
                            nc.scalar.activation(
                                ptd_s[:, r, q0:1024 - q0],
                                psS[:, q0:1024 - q0],
                                AF.Exp, scale=SCALE)
                            # staircase mask on each head's true diag block
                            for hh in (0, 1):
                                d0 = q0 if hh == 0 else 512
                                nc.vector.tensor_tensor(
                                    out=ptd_s[:, r, d0:d0 + P],
                                    in0=ptd_s[:, r, d0:d0 + P],
                                    in1=dmask_s[:, :], op=ALU.mult)
                        if j in pj:
                            tcx = pj[j]
                            proj_chunk(tcx,
                                       psmm_pool if tcx % 2 == 0 else psqp_pool)

                    # [V | 1]^T @ P^T per head: y~^T [65, 512] = unnormalized
                    # y^T rows + the softmax-denominator l row. Normalize by
                    # broadcasting 1/l across partitions (GpSimd) and multiply
                    # straight into the y^T pair-chunk buffer -- no transposes.
                    # The QK filler (or pair 3's 4th proj chunk) is emitted
                    # inside hh0's accumulation a few slots before the end,
                    # where the PE would otherwise stall on the last diag exps.
                    for hh in (0, 1):
                        nj = 4 * c + 4
                        if hh == 0 and nj <= 4 and fill_units:
                            for (m, th) in fill_units:
                                qkproj_half(m, th, psqp_pool)
                            fill_units = []
                        psyt = psmm_pool.tile([P, 512], dt.float32,
                                              name="psyt", tag="mm")
                        for j in range(nj):
                            r = j - 4 * c
                            if r < 0:
                                rhs = pt_s[:, j, hh, :]
                                out = psyt[0:65, :]
                            else:
                                # diagonal slot: only columns q >= 128r live
                                # (hh1's region sits left-shifted at 512..)
                                rhs = (ptd_s[:, r, P * r:512] if hh == 0
                                       else ptd_s[:, r, 512:1024 - P * r])
                                out = psyt[0:65, P * r:]
                            nc.tensor.matmul(
                                out, v_s[:, j, 2 * pair + hh, 0:65], rhs,
                                start=(j == 0), stop=(j == nj - 1))
                            if hh == 0 and j == nj - 5:
                                if fill_units:
                                    for (m, th) in fill_units:
                                        qkproj_half(m, th, psqp_pool)
                                    fill_units = []
                                elif late_proj is not None:
                                    proj_chunk(late_proj, psqp_pool)
                                    late_proj = None
                        # 1/l on ScalarE as exp(-ln(l)) (DVE RECIPROCAL is
                        # ~6.5 cyc/elem serial in the free dim), GpSimd
                        # partition-broadcast, DVE multiply into the y^T chunk.
                        lnl = ln_pool.tile([1, 512], dt.float32, name="lnl",
                                           tag="lnl")
                        nc.scalar.activation(lnl[:, :], psyt[64:65, :], AF.Ln)
                        linv = sp.tile([1, 512], dt.bfloat16, name="linv",
                                       tag="linv")
                        nc.scalar.activation(linv[:, :], lnl[:, :], AF.Exp,
                                             scale=-1.0)
                        lb = lb_pool.tile([64, 512], dt.bfloat16, name="lb",
                                          tag="lb")
                        nc.gpsimd.partition_broadcast(lb[:, :], linv[:, :],
                                                      channels=64)
                        nc.vector.tensor_tensor(
                            out=y2t_s[64 * hh:64 * (hh + 1), pair,
                                      512 * c:512 * (c + 1)],
                            in0=psyt[0:64, :], in1=lb[:, :], op=ALU.mult)

                # last block's output projection (nothing left to hide behind)
                if pair == NPAIR - 1:
                    for qi_loc in range(4):
                        tcx = 12 + qi_loc
                        proj_chunk(tcx,
                                   psmm_pool if tcx % 2 == 0 else psqp_pool)

    nc.compile()
    return nc


def _prep_inputs(x, w_attn, b_attn, w_proj, b_proj):
    """Host-side shard prep: per-core input dicts (core ci = b*2 + hg)."""
    x = np.asarray(x, dtype=np.float32)
    w_attn = np.asarray(w_attn, dtype=np.float32)
    b_attn = np.asarray(b_attn, dtype=np.float32)
    w_proj = np.asarray(w_proj, dtype=np.float32)
    b_proj = np.asarray(b_proj, dtype=np.float32)

    # diagonal staircase mask [tk, q]: valid iff q >= tk
    dmask = (np.arange(P)[None, :] >= np.arange(P)[:, None]).astype(BF16)

    in_maps = []
    for b in range(B):
        xT = np.ascontiguousarray(x[b].T).astype(BF16)       # [C, T]
        for hg in range(2):
            lo = hg * 512
            wqk = np.concatenate(
                [w_attn[:, lo:lo + 512], w_attn[:, 1024 + lo:1024 + lo + 512]],
                axis=1).astype(BF16)                          # [C, 1024]
            wv = w_attn[:, 2048 + lo:2048 + lo + 512].astype(BF16)
            wproj = w_proj[lo:lo + 512, :].astype(BF16)       # [512, C]
            bqk = np.stack(
                [b_attn[lo + P * m:lo + P * (m + 1)] for m in range(4)] +
                [b_attn[1024 + lo + P * m:1024 + lo + P * (m + 1)] for m in range(4)],
                axis=1).astype(np.float32)                    # [128, 8]
            bv = np.broadcast_to(b_attn[2048 + lo:2048 + lo + 512],
                                 (P, 512)).astype(np.float32)
            bp = b_proj if hg == 0 else np.zeros_like(b_proj)
            bproj = np.broadcast_to(bp, (P, C)).astype(np.float32)
            in_maps.append({
                "xT": xT, "wqk": wqk, "wv": wv, "wproj": wproj,
                "bqk": np.ascontiguousarray(bqk), "bv": np.ascontiguousarray(bv),
                "bproj": np.ascontiguousarray(bproj),
                "dmask": np.ascontiguousarray(dmask),
            })
    return in_maps


def kernel(x, w_attn, b_attn, w_proj, b_proj):
    global LAST_RESULT
    from concourse.bass_utils import run_bass_kernel_spmd

    if "nc" not in _CACHE:
        _CACHE["nc"] = _build_program()
    nc = _CACHE["nc"]

    in_maps = _prep_inputs(x, w_attn, b_attn, w_proj, b_proj)
    res = run_bass_kernel_spmd(nc, in_maps, core_ids=list(range(8)))
    LAST_RESULT = res

    out = np.zeros((B, T, C), dtype=np.float32)
    for b in range(B):
        out[b] = (res.results[2 * b]["out"].astype(np.float32) +
                  res.results[2 * b + 1]["out"].astype(np.float32))
    return out


# revision 28
# speedup vs baseline: 1.0877x; 1.0101x over previous
"""Causal self-attention (B=4, T=2048, C=1024, 16 heads) on 8 trn2 NeuronCores.

Sharding: core (b, hg) handles batch b (4) x head-group hg (2 groups of 8 heads).
Each core computes QKV projection for its batch restricted to its 8 heads,
flash-style causal attention, and the output projection restricted to its
heads' rows of w_proj -> a partial [T, C] output. Host sums the two partials
per batch (tensor-parallel unshard) and concatenates batches.

Key layout choices (all bf16 matmul inputs, fp32 PSUM accumulation):
  - x is fed pre-transposed per batch: xT [C, T], so Q^T/K^T [d, t] come
    straight out of the QKV matmuls (lhsT = w slice, rhs = xT).
  - Scores are computed TRANSPOSED: S^T[tk, q] = matmul with lhsT = K^T chunk,
    rhs = Q^T chunk; the pair's two heads sit on partition halves 0:64/64:128
    so their score matmuls row-tile onto disjoint PE row groups.
  - V carries an appended ones-column, so the A@V matmul (lhsT=[V|1], rhs=P^T)
    yields y~^T = [64 weighted-V rows | l row] x q in one [65, 512] pass.
    Since d is already on partitions, y~^T IS y^T unnormalized: the softmax
    division is done per q-COLUMN by broadcasting 1/l (ScalarE exp(-ln l)
    -> GpSimd partition_broadcast -> DVE multiply) straight into
    y2t [128 ch, T] pair-chunks. No PE transposes anywhere.
  - Causal masking: block-skipping, matmul column ranges narrowed to the valid
    q-range on diagonal slots, exp written only to the valid range of a
    dedicated diag buffer (whose sub-diagonal zeros are written once), and one
    [128,128] staircase mask multiplied into the true diagonal blocks.
  - Next pair's QKV projection matmuls are interleaved into the attention
    c-loop as PE filler while ScalarE works through the exps; pair 3
    interleaves the output-projection tail the same way. QK projection runs
    t4-pair-inner (same stationary weights for 2 consecutive matmuls) and the
    output projection co-inner (same y^T chunk for 2 matmuls) to cut
    LDWEIGHTS pressure.
"""

import numpy as np
import ml_dtypes

B, T, C, H, D = 4, 2048, 1024, 16, 64
P = 128
TC = T // P          # 16 t-chunks of 128
KC = C // P          # 8 contraction chunks of 128
NPAIR = 4            # head pairs per core (8 local heads)
SCALE = 0.125        # 1/sqrt(64)

_CACHE = {}
LAST_RESULT = None   # BassKernelResults of the most recent run (for test.py)

BF16 = ml_dtypes.bfloat16


def _build_program():
    import concourse.tile as tile
    import concourse.mybir as mybir
    from concourse import bacc

    dt = mybir.dt
    AF = mybir.ActivationFunctionType
    ALU = mybir.AluOpType

    # Bias the ACT-table placement pass so Exp and Ln both resolve to the
    # combined natural_log_exp_and_others set (index preserved): the pass
    # maps each function to the FIRST listed set containing it, and mixing
    # Exp/Ln would otherwise reload tables (~2.7us) on every transition.
    # Only the pass's view of set membership is masked; runtime table
    # contents and set ids are untouched.
    if not getattr(bacc, "_act_tables_patched", False):
        _orig_tables = bacc.get_activation_tables

        def _patched_tables(module_arch):
            out = {}
            for name, funcs in _orig_tables(module_arch).items():
                funcs = set(funcs)
                if name in ("exp_and_others", "natural_log"):
                    funcs.discard(AF.Exp)
                    funcs.discard(AF.Ln)
                out[name] = funcs
            return out

        bacc.get_activation_tables = _patched_tables
        bacc._act_tables_patched = True

    nc = bacc.Bacc("TRN2", target_bir_lowering=False, debug=False, num_devices=8)

    # ---- DRAM I/O ----
    xT_d = nc.dram_tensor("xT", [C, T], dt.bfloat16, kind="ExternalInput").ap()
    wqk_d = nc.dram_tensor("wqk", [C, 1024], dt.bfloat16, kind="ExternalInput").ap()
    wv_d = nc.dram_tensor("wv", [C, 512], dt.bfloat16, kind="ExternalInput").ap()
    wproj_d = nc.dram_tensor("wproj", [512, C], dt.bfloat16, kind="ExternalInput").ap()
    bqk_d = nc.dram_tensor("bqk", [P, 8], dt.float32, kind="ExternalInput").ap()
    bv_d = nc.dram_tensor("bv", [P, 512], dt.float32, kind="ExternalInput").ap()
    bproj_d = nc.dram_tensor("bproj", [P, C], dt.float32, kind="ExternalInput").ap()
    dmask_d = nc.dram_tensor("dmask", [P, P], dt.bfloat16, kind="ExternalInput").ap()
    out_d = nc.dram_tensor("out", [T, C], dt.bfloat16, kind="ExternalOutput").ap()

    with tile.TileContext(nc) as tc:
        with (
            tc.tile_pool(name="const", bufs=1) as cp,
            tc.tile_pool(name="outp", bufs=4) as op_pool,
            tc.tile_pool(name="small", bufs=3) as sp,
            tc.tile_pool(name="lnp", bufs=3) as ln_pool,
            tc.tile_pool(name="lbp", bufs=2) as lb_pool,
            tc.tile_pool(name="psqk", bufs=2, space="PSUM") as psqk_pool,
            tc.tile_pool(name="psmm", bufs=2, space="PSUM") as psmm_pool,
            tc.tile_pool(name="psqp", bufs=2, space="PSUM") as psqp_pool,
        ):
            # ---- static SBUF tensors ----
            xT_s = cp.tile([P, KC, T], dt.bfloat16, name="xT_s")
            wqk_s = cp.tile([P, KC, 1024], dt.bfloat16, name="wqk_s")
            wv_s = cp.tile([P, KC, 512], dt.bfloat16, name="wv_s")
            wproj_s = cp.tile([P, 4, C], dt.bfloat16, name="wproj_s")
            bqk_s = cp.tile([P, 8], dt.float32, name="bqk_s")
            bv_s = cp.tile([P, 512], dt.float32, name="bv_s")
            bproj_s = cp.tile([P, C], dt.float32, name="bproj_s")
            dmask_s = cp.tile([P, P], dt.bfloat16, name="dmask_s")
            qt_s = cp.tile([P, NPAIR, T], dt.bfloat16, name="qt_s")
            kt_s = cp.tile([P, NPAIR, T], dt.bfloat16, name="kt_s")
            # head stride 72 (144B) keeps each [V|1] slice 16B-aligned for LDW
            v_s = cp.tile([P, TC, 8, 72], dt.bfloat16, name="v_s")   # [t, tc, head, V|1|pad]
            y2t_s = cp.tile([P, NPAIR, T], dt.bfloat16, name="y2t_s")  # y^T [ch, t]
            pt_s = cp.tile([P, 12, 2, 512], dt.bfloat16, name="pt_s")   # exp(S^T) off-diag [slot,hh,q]
            # diag slots, flat [hh0 q | hh1 q-shifted]: hh1's valid region is
            # written at 512..1024-q0 so one contiguous exp covers both heads
            ptd_s = cp.tile([P, 4, 1024], dt.bfloat16, name="ptd_s")

            # ---- input DMAs (weights for the first compute first) ----
            xT_src = xT_d.rearrange("(o p) t -> p o t", p=P)
            wv_src = wv_d.rearrange("(o p) m -> p o m", p=P)
            wqk_src = wqk_d.rearrange("(o p) m -> p o m", p=P)
            nc.sync.dma_start(wv_s[:, 0:2, :], wv_src[:, 0:2, :])
            nc.sync.dma_start(wv_s[:, 2:4, :], wv_src[:, 2:4, :])
            nc.sync.dma_start(xT_s[:, :, 0:128], xT_src[:, :, 0:128])
            nc.sync.dma_start(wv_s[:, 4:6, :], wv_src[:, 4:6, :])
            nc.sync.dma_start(wv_s[:, 6:8, :], wv_src[:, 6:8, :])
            nc.sync.dma_start(xT_s[:, :, 128:256], xT_src[:, :, 128:256])
            nc.sync.dma_start(bv_s[:], bv_d)
            nc.sync.dma_start(xT_s[:, :, 256:512], xT_src[:, :, 256:512])
            nc.sync.dma_start(wqk_s[:, 0:4, :], wqk_src[:, 0:4, :])
            nc.sync.dma_start(xT_s[:, :, 512:768], xT_src[:, :, 512:768])
            nc.sync.dma_start(wqk_s[:, 4:8, :], wqk_src[:, 4:8, :])
            nc.sync.dma_start(bqk_s[:], bqk_d)
            for q8 in range(3, 8):
                nc.sync.dma_start(xT_s[:, :, 256 * q8:256 * (q8 + 1)],
                                  xT_src[:, :, 256 * q8:256 * (q8 + 1)])
            nc.sync.dma_start(dmask_s[:], dmask_d)
            nc.sync.dma_start(wproj_s[:], wproj_d.rearrange("(o p) m -> p o m", p=P))
            nc.sync.dma_start(bproj_s[:], bproj_d)

            # ones column of V~; zero the diag P^T buffer once (sub-diagonal
            # regions are never written by the partial exps, so zeros persist)
            nc.vector.memset(v_s[:, :, :, 64:65], 1.0)
            nc.vector.memset(ptd_s[:], 0.0)

            # ---- V projection: v[t, ch] for all 8 heads (512 cols) ----
            for tcx in range(TC):
                psv = psmm_pool.tile([P, 512], dt.float32, name="psv", tag="mm")
                for k in range(KC):
                    nc.tensor.matmul(psv[:, :],
                                     xT_s[:, k, P * tcx:P * (tcx + 1)],
                                     wv_s[:, k, :],
                                     start=(k == 0), stop=(k == KC - 1))
                nc.vector.tensor_add(
                    out=v_s[:, tcx, :, 0:64],
                    in0=psv[:, :].rearrange("a (h d) -> a h d", h=8),
                    in1=bv_s[:, :].rearrange("a (h d) -> a h d", h=8),
                )

            # ---- helper emitters ----
            def pool_tag(pool):
                return "mm" if pool is psmm_pool else "qp"

            def qkproj_half(m, th, pool):
                """Two [128 out-ch, 512 t] tiles of the Q^T/K^T projection,
                k-inner with both t4 tiles per k so each stationary weight
                chunk serves 2 consecutive matmuls."""
                dst = qt_s if m < 4 else kt_s
                t4s = (2 * th, 2 * th + 1)
                pss = [pool.tile([P, 512], dt.float32, name=f"psq{i}",
                                 tag=pool_tag(pool)) for i in range(2)]
                for k in range(KC):
                    for i, t4 in enumerate(t4s):
                        nc.tensor.matmul(pss[i][:, :],
                                         wqk_s[:, k, P * m:P * (m + 1)],
                                         xT_s[:, k, 512 * t4:512 * (t4 + 1)],
                                         start=(k == 0), stop=(k == KC - 1))
                for i, t4 in enumerate(t4s):
                    nc.vector.tensor_scalar(
                        out=dst[:, m % 4, 512 * t4:512 * (t4 + 1)],
                        in0=pss[i][:, :], scalar1=bqk_s[:, m:m + 1], scalar2=None,
                        op0=ALU.add)

            def proj_chunk(tcx, pool):
                """Output projection for one 128-row t-chunk, co-inner so each
                y^T chunk serves 2 consecutive matmuls."""
                pss = [pool.tile([P, 512], dt.float32, name=f"psp{co}",
                                 tag=pool_tag(pool)) for co in range(2)]
                for cc in range(4):
                    for co in range(2):
                        nc.tensor.matmul(pss[co][:, :],
                                         y2t_s[:, cc, P * tcx:P * (tcx + 1)],
                                         wproj_s[:, cc, 512 * co:512 * (co + 1)],
                                         start=(cc == 0), stop=(cc == 3))
                for co in range(2):
                    ot = op_pool.tile([P, 512], dt.bfloat16, name="ot", tag="ot")
                    nc.vector.tensor_add(out=ot[:, :], in0=pss[co][:, :],
                                         in1=bproj_s[:, 512 * co:512 * (co + 1)])
                    nc.sync.dma_start(
                        out_d[P * tcx:P * (tcx + 1), 512 * co:512 * (co + 1)], ot[:, :])

            # ---- pair 0 projection upfront; later pairs interleave ----
            for m in (0, 4):
                for th in range(2):
                    qkproj_half(m, th, psqp_pool)

            for pair in range(NPAIR):
                nxt = ([(m, th) for m in (pair + 1, 5 + pair) for th in range(2)]
                       if pair < NPAIR - 1 else [])
                for ci, c in enumerate(range(4)):   # q chunk of 512
                    # pair 3 has no QK filler; spread the previous block's
                    # output projection across its scores phase instead
                    if pair == NPAIR - 1 and c > 0:
                        nslots = 4 * c + 4
                        pj = {(i + 1) * nslots // 5: 4 * (c - 1) + i
                              for i in range(3)}
                        late_proj = 4 * (c - 1) + 3   # odd tcx -> psqp pool
                    else:
                        pj = {}
                        late_proj = None
                    fill_units = list(nxt[ci:ci + 1])
                    for j in range(4 * c + 4):          # tk chunk (slot)
                        r = j - 4 * c                   # >= 0 on diagonal slots
                        q0 = P * r if r >= 0 else 0     # skip masked cols
                        psS = psqk_pool.tile([P, 1024], dt.float32, name="psS",
                                             tag="psqk")
                        for hh in (0, 1):
                            base = 64 * hh
                            # hh1's diag output is left-shifted by q0 so the
                            # two heads' valid score regions abut at 512
                            nc.tensor.matmul(
                                psS[:, q0:512] if hh == 0
                                else psS[:, 512:1024 - q0],
                                kt_s[base:base + 64, pair, P * j:P * (j + 1)],
                                qt_s[base:base + 64, pair,
                                     512 * c + q0:512 * (c + 1)],
                                start=True, stop=True)
                        # exp( S^T * scale ), fp32 psum -> bf16 sbuf
                        if r < 0:
                            nc.scalar.activation(pt_s[:, j, :, :], psS[:, :],
                                                 AF.Exp, scale=SCALE)
                        else:
                            nc.scalar.activation(
                                ptd_s[:, r, q0:1024 - q0],
                                psS[:, q0:1024 - q0],
                                AF.Exp, scale=SCALE)
                            # staircase mask on each head's true diag block
                            for hh in (0, 1):
                                d0 = q0 if hh == 0 else 512
                                nc.vector.tensor_tensor(
                                    out=ptd_s[:, r, d0:d0 + P],
                                    in0=ptd_s[:, r, d0:d0 + P],
                                    in1=dmask_s[:, :], op=ALU.mult)
                        if j in pj:
                            tcx = pj[j]
                            proj_chunk(tcx,
                                       psmm_pool if tcx % 2 == 0 else psqp_pool)

                    # [V | 1]^T @ P^T per head: y~^T [65, 512] = unnormalized
                    # y^T rows + the softmax-denominator l row. Normalize by
                    # broadcasting 1/l across partitions (GpSimd) and multiply
                    # straight into the y^T pair-chunk buffer -- no transposes.
                    # The QK filler (or pair 3's 4th proj chunk) is emitted
                    # inside hh0's accumulation a few slots before the end,
                    # where the PE would otherwise stall on the last diag exps.
                    for hh in (0, 1):
                        nj = 4 * c + 4
                        if hh == 0 and nj <= 4 and fill_units:
                            for (m, th) in fill_units:
                                qkproj_half(m, th, psqp_pool)
                            fill_units = []
                        psyt = psmm_pool.tile([P, 512], dt.float32,
                                              name="psyt", tag="mm")
                        for j in range(nj):
                            r = j - 4 * c
                            if r < 0:
                                rhs = pt_s[:, j, hh, :]
                                out = psyt[0:65, :]
                            else:
                                # diagonal slot: only columns q >= 128r live
                                # (hh1's region sits left-shifted at 512..)
                                rhs = (ptd_s[:, r, P * r:512] if hh == 0
                                       else ptd_s[:, r, 512:1024 - P * r])
                                out = psyt[0:65, P * r:]
                            nc.tensor.matmul(
                                out, v_s[:, j, 2 * pair + hh, 0:65], rhs,
                                start=(j == 0), stop=(j == nj - 1))
                            if hh == 0 and j == nj - 5:
                                if fill_units:
                                    for (m, th) in fill_units:
                                        qkproj_half(m, th, psqp_pool)
                                    fill_units = []
                                elif late_proj is not None:
                                    proj_chunk(late_proj, psqp_pool)
                                    late_proj = None
                        # 1/l on ScalarE as exp(-ln(l)) (DVE RECIPROCAL is
                        # ~6.5 cyc/elem serial in the free dim), GpSimd
                        # partition-broadcast, DVE multiply into the y^T chunk.
                        lnl = ln_pool.tile([1, 512], dt.float32, name="lnl",
                                           tag="lnl")
                        nc.scalar.activation(lnl[:, :], psyt[64:65, :], AF.Ln)
                        linv = sp.tile([1, 512], dt.bfloat16, name="linv",
                                       tag="linv")
                        nc.scalar.activation(linv[:, :], lnl[:, :], AF.Exp,
                                             scale=-1.0)
                        lb = lb_pool.tile([64, 512], dt.bfloat16, name="lb",
                                          tag="lb")
                        nc.gpsimd.partition_broadcast(lb[:, :], linv[:, :],
                                                      channels=64)
                        nc.vector.tensor_tensor(
                            out=y2t_s[64 * hh:64 * (hh + 1), pair,
                                      512 * c:512 * (c + 1)],
                            in0=psyt[0:64, :], in1=lb[:, :], op=ALU.mult)

                # last block's output projection (nothing left to hide behind)
                if pair == NPAIR - 1:
                    for qi_loc in range(4):
                        tcx = 12 + qi_loc
                        proj_chunk(tcx,
                                   psmm_pool if tcx % 2 == 0 else psqp_pool)

    nc.compile()
    return nc


def _prep_inputs(x, w_attn, b_attn, w_proj, b_proj):
    """Host-side shard prep: per-core input dicts (core ci = b*2 + hg)."""
    x = np.asarray(x, dtype=np.float32)
    w_attn = np.asarray(w_attn, dtype=np.float32)
    b_attn = np.asarray(b_attn, dtype=np.float32)
    w_proj = np.asarray(w_proj, dtype=np.float32)
    b_proj = np.asarray(b_proj, dtype=np.float32)

    # diagonal staircase mask [tk, q]: valid iff q >= tk
    dmask = (np.arange(P)[None, :] >= np.arange(P)[:, None]).astype(BF16)

    in_maps = []
    for b in range(B):
        xT = np.ascontiguousarray(x[b].T).astype(BF16)       # [C, T]
        for hg in range(2):
            lo = hg * 512
            wqk = np.concatenate(
                [w_attn[:, lo:lo + 512], w_attn[:, 1024 + lo:1024 + lo + 512]],
                axis=1).astype(BF16)                          # [C, 1024]
            wv = w_attn[:, 2048 + lo:2048 + lo + 512].astype(BF16)
            wproj = w_proj[lo:lo + 512, :].astype(BF16)       # [512, C]
            bqk = np.stack(
                [b_attn[lo + P * m:lo + P * (m + 1)] for m in range(4)] +
                [b_attn[1024 + lo + P * m:1024 + lo + P * (m + 1)] for m in range(4)],
                axis=1).astype(np.float32)                    # [128, 8]
            bv = np.broadcast_to(b_attn[2048 + lo:2048 + lo + 512],
                                 (P, 512)).astype(np.float32)
            bp = b_proj if hg == 0 else np.zeros_like(b_proj)
            bproj = np.broadcast_to(bp, (P, C)).astype(np.float32)
            in_maps.append({
                "xT": xT, "wqk": wqk, "wv": wv, "wproj": wproj,
                "bqk": np.ascontiguousarray(bqk), "bv": np.ascontiguousarray(bv),
                "bproj": np.ascontiguousarray(bproj),
                "dmask": np.ascontiguousarray(dmask),
            })
    return in_maps


def kernel(x, w_attn, b_attn, w_proj, b_proj):
    global LAST_RESULT
    from concourse.bass_utils import run_bass_kernel_spmd

    if "nc" not in _CACHE:
        _CACHE["nc"] = _build_program()
    nc = _CACHE["nc"]

    in_maps = _prep_inputs(x, w_attn, b_attn, w_proj, b_proj)
    res = run_bass_kernel_spmd(nc, in_maps, core_ids=list(range(8)))
    LAST_RESULT = res

    out = np.zeros((B, T, C), dtype=np.float32)
    for b in range(B):
        out[b] = (res.results[2 * b]["out"].astype(np.float32) +
                  res.results[2 * b + 1]["out"].astype(np.float32))
    return out
